# revision 83
# baseline (speedup 1.0000x reference)
"""Trainium2 Bass kernel for nn_AttentionBlock (causal single-head attention,
8192 tokens, qk-dim 16, v-dim 128, 1x1-conv projections with positional enc).

Sharding: striped query-parallel over 8 cores. Core m owns query tokens
{m, m+8, ..., m+8184} (1024 queries) -- perfectly balanced causal work AND an
identical instruction stream on every core (required: one NEFF, SPMD).

v2: fp8(e4m3) datapath for the projections and attn@V.
  - x and the 1x1-conv weights are shipped in fp8e4m3; Wq/Wk are scaled x16
    host-side (their native ~0.06 range would hit e4m3 denormals) and the
    compensating 1/256 -- together with the 1/sqrt(259) score scale -- is
    folded into the exp activation's free affine `scale`.
  - Projections contract 264 channels as one DoubleRow matmul over the two
    128-channel subtiles (0.5 cyc/row) plus a regular fp8 tail matmul over
    the 8 remaining rows (pos-enc, ones/bias, pad).
  - exp output is written as fp8 directly by ScalarE; attn@V runs DoubleRow
    (two 128-key blocks per matmul, 256-key contraction), with V stored fp8
    at stride VSTR=144 (DoubleRow weight-step must be 16B-aligned).
  - Scores stay bf16/fp32 (S matmuls bf16, PSUM fp32, exp on ScalarE).
  - Precision guard: super-group 0 of pair 0 (key blocks 0..3 x the first
    256 queries, i.e. every query's first 512 keys on this core) runs the
    old bf16 path -- small-softmax rows (few keys) would otherwise expose
    raw fp8 V quantization in the output.
  - exp is batched 2 score-groups per activation instruction (PSUM source
    spanning 2 banks) to amortize ScalarE's fixed per-instruction cost;
    the causal column cut is applied at super-group granularity and the
    multiplicative mask (fp8, generated on-device) runs on GPSIMD.
  - o_t accumulates lo+hi query-subblocks in ONE PSUM bank as a single
    accumulation group (start=True only on the pair's very first attn@V
    matmul -- a start mid-bank would clear the sibling's has_written bits).
"""

import os
import numpy as np

P = 128
NTOK = 8192
KC, VC = 16, 128
NCORES = 8
NQ = NTOK // NCORES       # 1024 queries per core
QSUBS = NQ // P           # 8
NPAIR = QSUBS // 2        # 4 query-subblock pairs
VSTR = 144                # fp8 V row stride: 128 v + 1 ones + 15 zero pad
CCH = 264                 # channels: 259 x + 3 pos + 1 ones + 1 zero pad
CPAD = 512                # channel rows padded to 4 x 128 subtiles (zeros)
NGRP = NPAIR              # 4 DMA groups of 2048 tokens (4 chunks of 512)
WQR = 128                 # ww cols 0:128   = Wq(x16) replicated at 0/32/64/96
WKO = 128                 # ww cols 128:144 = Wk(x16)
WVO = 144                 # ww cols 144:288 = Wv | ones | zero pad
CW = 288
SCALE = float(1.0 / (256.0 * np.sqrt(259.0)))

LAST_RESULTS = None       # BassKernelResults of the most recent run (for test.py)

_CACHE = {}


def _build_bass():
    import concourse.mybir as mybir
    import concourse.tile as tile
    from concourse import bacc

    f32 = mybir.dt.float32
    bf16 = mybir.dt.bfloat16
    fp8 = mybir.dt.float8e4
    AF = mybir.ActivationFunctionType
    ALU = mybir.AluOpType
    DR = mybir.MatmulPerfMode.DoubleRow

    nc = bacc.Bacc("TRN2", target_bir_lowering=False, debug=False,
                   num_devices=NCORES)

    xq_d = nc.dram_tensor("xq", [CPAD, NQ], fp8, kind="ExternalInput").ap()
    xkv_d = nc.dram_tensor("xkv", [CPAD, NTOK], fp8, kind="ExternalInput").ap()
    ww_d = nc.dram_tensor("ww", [CPAD, CW], fp8, kind="ExternalInput").ap()
    mask_d = nc.dram_tensor("mask8", [P, 8 * 2 * 64], fp8,
                            kind="ExternalInput").ap()
    xb0_d = nc.dram_tensor("xb0", [384, 512], bf16, kind="ExternalInput").ap()
    wvb_d = nc.dram_tensor("wvb", [384, VSTR], bf16,
                           kind="ExternalInput").ap()
    mask0_d = nc.dram_tensor("mask0", [P, 2, 2, 64], bf16,
                             kind="ExternalInput").ap()
    y_d = nc.dram_tensor("y", [NQ, VC], f32, kind="ExternalOutput").ap()

    with tile.TileContext(nc) as tc:
        with (
            tc.tile_pool(name="const", bufs=1) as const,
            tc.tile_pool(name="xpool", bufs=3) as xpool,
            tc.tile_pool(name="work", bufs=5) as work,
            tc.tile_pool(name="small", bufs=8) as small,
            tc.tile_pool(name="ps_s", bufs=2, space="PSUM") as ps_s,
            tc.tile_pool(name="ps_o", bufs=2, space="PSUM") as ps_o,
            tc.tile_pool(name="ps_kq", bufs=1, space="PSUM") as ps_kq,
            tc.tile_pool(name="ps_v", bufs=1, space="PSUM") as ps_v,
        ):
          # ---- body (emitted KREPEAT times for device-time measurement) ----
          for _rep in range(int(os.environ.get("KREPEAT", "1"))):
            # ---- persistent SBUF tensors ----
            ww_sb = const.tile([P, 4, CW], fp8)
            xq_sb = const.tile([P, 4, NQ], fp8)
            mask_sb = const.tile([P, 8, 2, 64], fp8)
            mask0_sb = const.tile([P, 2, 2, 64], bf16)
            qt_sb = const.tile([P, NQ], bf16)
            kt_sb = const.tile([P, NGRP, 512], bf16)
            v_sb = const.tile([P, 32, 2, VSTR], fp8)
            v0_sb = const.tile([P, 4, VSTR], bf16)
            xb0_sb = const.tile([P, 3, 512], bf16)
            wvb_sb = const.tile([P, 3, VSTR], bf16)

            # ---- DMA ordering (all SP-issued, 565ns per issue): everything
            # the first exp needs goes first -- ww, xq half 0, token group 0
            # piece 0 -- then the rest in first-use order. The channel axis
            # is zero-padded host-side to 4x128 subtiles so every tensor
            # loads with ONE descriptor set and the tail contraction can run
            # DoubleRow. Masks come precomputed from the host (cheaper as
            # DMA bytes than as DVE/Pool compute).
            nc.sync.dma_start(ww_sb[:],
                              ww_d.rearrange("(c p) m -> p c m", p=P))

            def dma_xq_piece(c):
                nc.sync.dma_start(
                    xq_sb[:, :, 256 * c:256 * (c + 1)],
                    xq_d[:, 256 * c:256 * (c + 1)]
                    .rearrange("(c p) n -> p c n", p=P))
            wq2 = ww_sb[:, 0:2, 0:WQR]
            wk2 = ww_sb[:, 0:2, WKO:WKO + KC]
            wv2 = ww_sb[:, 0:2, WVO:WVO + VC]
            wqt = ww_sb[:, 2:4, 0:WQR]
            wkt = ww_sb[:, 2:4, WKO:WKO + KC]
            wvt = ww_sb[:, 2:4, WVO:WVO + VC]
            grp_tiles = {}

            def ensure_group_dma(g):
                if g in grp_tiles or g >= NGRP:
                    return
                xg = xpool.tile([P, 4, 2048], fp8, tag="xg", name=f"xg{g}")
                pieces = {0: 4, 1: 2}.get(g, 1)
                for hh in range(pieces):
                    w = 2048 // pieces
                    sl = slice(2048 * g + w * hh, 2048 * g + w * (hh + 1))
                    cs = slice(w * hh, w * (hh + 1))
                    nc.sync.dma_start(
                        xg[:, :, cs],
                        xkv_d[:, sl].rearrange("(c p) n -> p c n", p=P))
                grp_tiles[g] = xg

            # ---- PE p-state warm-up: the cost model ramps the PE clock
            # (0.65 -> 1.2 -> 2.4 GHz) based on how long the engine has been
            # continuously busy; real work only starts ~4.5us in (DMA
            # latency), so a chain of throwaway matmuls on never-read SBUF
            # keeps the PE busy from t=0 and the projections start warm ----
            warm_src = const.tile([P, 512], bf16, name="warm_src")
            nc.gpsimd.memset(warm_src[:], 0.0)
            warm_ps = ps_kq.tile([P, 512], f32, tag="kq", name="warm_ps")
            for _w in range(7):
                nc.tensor.matmul(warm_ps[:], warm_src[:, 0:128],
                                 warm_src[:], start=True, stop=True,
                                 skip_group_check=True)

            # SP issue order: group-0 piece 0 first (gates K0 -> first exp),
            # then xq piece 0 (gates Q proj), then first-use order
            xg0 = xpool.tile([P, 4, 2048], fp8, tag="xg", name="xg0")
            nc.sync.dma_start(
                xg0[:, :, 0:512],
                xkv_d[:, 0:512].rearrange("(c p) n -> p c n", p=P))
            grp_tiles[0] = xg0
            dma_xq_piece(0)
            nc.sync.dma_start(
                xg0[:, :, 512:1024],
                xkv_d[:, 512:1024].rearrange("(c p) n -> p c n", p=P))
            nc.sync.dma_start(mask0_sb[:], mask0_d)
            dma_xq_piece(1)
            nc.sync.dma_start(
                xg0[:, :, 1024:1536],
                xkv_d[:, 1024:1536].rearrange("(c p) n -> p c n", p=P))
            nc.sync.dma_start(
                xg0[:, :, 1536:2048],
                xkv_d[:, 1536:2048].rearrange("(c p) n -> p c n", p=P))
            nc.sync.dma_start(xb0_sb[:],
                              xb0_d.rearrange("(c p) n -> p c n", p=P))
            nc.sync.dma_start(wvb_sb[:],
                              wvb_d.rearrange("(c p) n -> p c n", p=P))
            dma_xq_piece(2)
            dma_xq_piece(3)
            nc.sync.dma_start(
                mask_sb[:], mask_d.rearrange("p (d u n) -> p d u n", d=8, u=2))
            ensure_group_dma(1)
            # softmax-denominator ones column of every V tile (col 128)
            nc.gpsimd.memset(v_sb[:, :, :, VC:VC + 1], 1.0)
            nc.gpsimd.memset(v0_sb[:, :, VC:VC + 1], 1.0)

            # ---- ACT table warm-up: load the exp table set while the input
            # DMAs stream in (the real first exp would otherwise eat ~1.3us
            # on the critical path) ----
            warm_sb = small.tile([P, 1], f32, tag="recip")
            nc.scalar.activation(warm_sb[:], ww_sb[:, 0, 0:1], AF.Exp)

            # ---- Q projection: qt [16, 1024] replicated at partition bases
            # 0/32/64/96 (Wq is host-replicated into 4 column blocks).
            # Half 1 (query cols 512:1024, first needed by pair 2) is
            # deferred to pair 1 so it never gates pair-0's K projection ----
            def emit_q_proj(c):
                # pieces 0/1 borrow the ps_o ring (its o-tiles see no writes
                # until attn@V several fronts later), keeping the single-
                # buffer ps_v ring free for V0 -- otherwise V0's matmuls
                # queue behind the qt0 copy at the PE FIFO head
                csl = slice(256 * c, 256 * (c + 1))
                pool = ps_o if c < 2 else ps_v
                q_ps = pool.tile([P, 2, VSTR], f32, tag="o" if c < 2 else "v",
                                 name=f"qps{c}")
                qv = q_ps[:].rearrange("p a b -> p (a b)")[:, 0:256]
                nc.tensor.matmul(
                    qv, wq2, xq_sb[:, 0:2, csl],
                    start=True, stop=False, perf_mode=DR,
                    skip_group_check=True,
                )
                nc.tensor.matmul(
                    qv, wqt, xq_sb[:, 2:4, csl],
                    start=False, stop=True, perf_mode=DR,
                    skip_group_check=True,
                )
                nc.vector.tensor_copy(qt_sb[:, csl], qv)

            def emit_k_chunk(kq, p, j):
                """K-projection matmuls for chunk 4p+j at partition base 32j.

                DoubleRow requires dst partition base 0 (walrus
                s3d3_mm_valid_dst_partition), so only the j=0 chunk runs DR;
                the others use regular fp8 matmuls per 128-channel subtile.
                """
                xg = grp_tiles[p]
                tsl = slice(j * 512, (j + 1) * 512)
                if j == 0:
                    nc.tensor.matmul(
                        kq[0:KC, :], wk2, xg[:, 0:2, tsl],
                        start=True, stop=False, perf_mode=DR,
                        tile_position=(0, 0), skip_group_check=True,
                    )
                    nc.tensor.matmul(
                        kq[0:KC, :], wkt, xg[:, 2:4, tsl],
                        start=False, stop=True, perf_mode=DR,
                        tile_position=(0, 0), skip_group_check=True,
                    )
                    return
                for ci in range(3):
                    kk = P if ci < 2 else 8
                    nc.tensor.matmul(
                        kq[32 * j:32 * j + KC, :],
                        ww_sb[0:kk, ci, WKO:WKO + KC],
                        xg[0:kk, ci, tsl],
                        start=(ci == 0), stop=(ci == 2),
                        tile_position=(0, 32 * j), skip_group_check=True,
                    )

            def emit_v_chunk(p, j):
                """V-projection for the 4 token tiles of chunk 4p+j.
                4 x 128-col tiles pack into ONE PSUM bank (the softmax-
                denominator ones column is a one-time memset in v_sb, not
                part of the projection), so the whole chunk moves to SBUF
                with a single DVE copy."""
                xg = grp_tiles[p]
                v_ps = ps_v.tile([P, 4, VC], f32, tag="v", name=f"vps{p}{j}")
                for tl in range(4):
                    tsl = slice(j * 512 + tl * P, j * 512 + (tl + 1) * P)
                    nc.tensor.matmul(
                        v_ps[:, tl, :], xg[:, 0:2, tsl], wv2,
                        start=True, stop=False, perf_mode=DR,
                        skip_group_check=True,
                    )
                    nc.tensor.matmul(
                        v_ps[:, tl, :], xg[:, 2:4, tsl], wvt,
                        start=False, stop=True, perf_mode=DR,
                        skip_group_check=True,
                    )
                kk = 8 * p + 2 * j
                # (GPSIMD cannot read PSUM, so this stays on DVE)
                nc.vector.tensor_copy(
                    v_sb[:, kk:kk + 2, :, 0:VC], v_ps[:])

            def emit_v0_proj():
                """bf16 V projection for token tiles 0..3 (the fp8-projected
                V would expose its ~3% error directly in the outputs of the
                first, few-key softmax rows; tiles 0..3 cover every row the
                bf16 attn@V fallback path reads). Ones column comes from a
                memset; 4 x 128-col tiles fit one PSUM bank."""
                v0_ps = ps_v.tile([P, 4, VC], f32, tag="v", name="v0ps")
                for tl in range(4):
                    tsl = slice(tl * P, (tl + 1) * P)
                    for ci in range(3):
                        kk = P if ci < 2 else 8
                        nc.tensor.matmul(
                            v0_ps[:, tl, :], xb0_sb[0:kk, ci, tsl],
                            wvb_sb[0:kk, ci, 0:VC],
                            start=(ci == 0), stop=(ci == 2),
                            skip_group_check=True,
                        )
                nc.vector.tensor_copy(v0_sb[:, :, 0:VC], v0_ps[:])

            def emit_sg_front(p, s):
                """S matmuls + batched exp (+ causal mask for diagonal
                super-groups) for super-group s of pair p; returns the a tile.

                The mask multiply only touches [cut, cut+64): each of the 4
                key blocks has a 16-column staircase window there; above it
                the mask is exactly 1.0, below it the column cut applies.
                """
                nsg = 4 * p + 4
                diag = s >= nsg - 4
                dd = s - (nsg - 4)
                cut = 64 * dd if diag else 0
                bf16_path = (p == 0 and s == 0)
                s_ps = ps_s.tile([P, 2, 2, 256], f32, tag="s")
                for gg in range(2):
                    for u in range(2):
                        kb = 4 * s + 2 * gg + u
                        c = kb // 4
                        b = 32 * (c % 4)
                        klhs = kt_sb[b:b + KC, c // 4,
                                     128 * (kb % 4):128 * (kb % 4) + 128]
                        qrhs = qt_sb[b:b + KC, 256 * p + cut:256 * p + 256]
                        nc.tensor.matmul(
                            s_ps[:, gg, u, cut:256], klhs, qrhs,
                            start=True, stop=True,
                            tile_position=(b, 0),
                        )
                a_dt = bf16 if bf16_path else fp8
                a_sb = work.tile([P, 2, 2, 256], a_dt,
                                 tag="a16" if bf16_path else "a8")
                # attn@V reads fixed [0:128) / [128:256) query windows of a;
                # zero the sub-cut region it would read but exp won't write
                if diag and dd == 1:
                    nc.gpsimd.memset(a_sb[:, :, :, 0:64], 0.0)
                elif diag and dd == 3:
                    nc.gpsimd.memset(a_sb[:, :, :, 128:192], 0.0)
                nc.scalar.activation(a_sb[:, :, :, cut:256],
                                     s_ps[:, :, :, cut:256],
                                     AF.Exp, scale=SCALE)
                if diag:
                    # the shipped mask tiles hold ONLY the 64-col staircase
                    # window [cut, cut+64) of each (dloc, u)
                    msk = mask0_sb if bf16_path else mask_sb[:, 2 * dd:2 * dd + 2]
                    hi = min(256, cut + 64)
                    eng = nc.vector if (p, s) == (NPAIR - 1, nsg - 1) \
                        else nc.gpsimd
                    eng.tensor_mul(a_sb[:, :, :, cut:hi],
                                   a_sb[:, :, :, cut:hi],
                                   msk[:, :, :, 0:hi - cut])
                return a_sb

            def emit_sg_av(p, s, o_t, a_sb):
                """attn@V for super-group s of pair p (fp8 DoubleRow, 2 key
                blocks per matmul; bf16 fallback for pair-0 super-group 0)."""
                nsg = 4 * p + 4
                bf16_path = (p == 0 and s == 0)
                nmm_lo_g = 8 * p + 4      # first group with no lo contribution
                if bf16_path:
                    for gg in range(2):
                        for u in range(2):
                            kb = 2 * gg + u
                            nc.tensor.matmul(
                                o_t[:, 0, :], a_sb[:, gg, u, 0:128],
                                v0_sb[:, kb, :],
                                start=(kb == 0), stop=False,
                                skip_group_check=True,
                            )
                            nc.tensor.matmul(
                                o_t[:, 1, :], a_sb[:, gg, u, 128:256],
                                v0_sb[:, kb, :],
                                start=False, stop=False,
                                skip_group_check=True,
                            )
                    return
                for gg in range(2):
                    g = 2 * s + gg
                    vr = v_sb[:, g, :, :]
                    if g < nmm_lo_g:
                        nc.tensor.matmul(
                            o_t[:, 0, :], a_sb[:, gg, :, 0:128], vr,
                            start=(p > 0 and s == 0 and gg == 0),
                            stop=False, perf_mode=DR,
                            skip_group_check=True,
                        )
                    last = (s == last_av_s[p] and gg == 1)
                    nc.tensor.matmul(
                        o_t[:, 1, :], a_sb[:, gg, :, 128:256], vr,
                        start=False, stop=last, perf_mode=DR,
                        skip_group_check=True,
                    )

            def emit_norm(p, half, o_t):
                qs = 2 * p + half
                recip_sb = small.tile([P, 1], f32, tag="recip")
                if p == 0:
                    # only pair 0 can see an all-masked row (denominator 0)
                    nc.vector.tensor_scalar_add(
                        recip_sb[:], o_t[:, half, VC:VC + 1], 1e-30)
                    nc.vector.reciprocal(recip_sb[:], recip_sb[:])
                else:
                    nc.vector.reciprocal(recip_sb[:], o_t[:, half, VC:VC + 1])
                on_sb = small.tile([P, VC], f32, tag="on")
                nc.vector.tensor_scalar_mul(
                    on_sb[:], o_t[:, half, 0:VC], recip_sb[:])
                nc.sync.dma_start(y_d[qs * P:(qs + 1) * P, :], on_sb[:])

            # ---- chunk-major wavefront schedule. Super-group (p, s) needs
            # exactly global token-chunk s (keys 512s..512s+511), which is
            # produced by pair s//4's projection. So: 16 rounds, one chunk
            # each; after producing chunk c run its diagonal super-group
            # (c//4, c), its off-diagonal partner (c//4+1, c), and one
            # backlog "filler" super-group of a later pair whose chunk is
            # long since resident. This keeps ScalarE fed with a uniform
            # 2-3 exps per round (the old pair-sequential order crammed
            # pair 3's 16 super-groups at the end) and caps live o_t
            # accumulators at 2 (ps_o bufs=2; ps_v only needs 1 buffer
            # since V chunks are now one per round). attn@V still trails
            # by 3 fronts; the lo/hi norm trigger conditions are unchanged
            # because each pair's last lo / last av front is still its
            # s == nsg-3 / nsg-1 diagonal. ----
            o_tiles = {}
            fronts = []       # (p, s, pre_hook)
            last_av_s = {0: 3, 1: 7, 2: 11, 3: 15}  # last-PROCESSED sg per pair

            def chunk_hook(c):
                pc, j = c // 4, c % 4
                def hook():
                    if j == 0:
                        ensure_group_dma(pc + 1)
                        ensure_group_dma(pc + 2)
                        grp_tiles[("kq", pc)] = ps_kq.tile(
                            [P, 512], f32, tag="kq", name=f"kq{pc}")
                    kq = grp_tiles[("kq", pc)]
                    emit_k_chunk(kq, pc, j)
                    nc.vector.tensor_copy(
                        kt_sb[32 * j:32 * j + KC, pc, :],
                        kq[32 * j:32 * j + KC, :])
                    if c == 0:
                        emit_q_proj(0)
                    emit_v_chunk(pc, j)
                    if c == 0:
                        emit_q_proj(1)
                    elif c == 1:
                        emit_v0_proj()
                    elif c == 2:
                        emit_q_proj(2)
                    elif c == 6:
                        emit_q_proj(3)
                return hook

            def alloc_o(p):
                def hook():
                    o_tiles[p] = ps_o.tile([P, 2, VSTR], f32, tag="o",
                                           name=f"o{p}")
                return hook

            seen_pairs = set()

            def add_front(p, s, hooks):
                if p not in seen_pairs:
                    seen_pairs.add(p)
                    hooks = list(hooks) + [alloc_o(p)]
                fronts.append((p, s, hooks))

            # backlog fillers, front-loaded: pair-3's fills are all lo-half
            # super-groups, so they must finish well before the end or the
            # lo norm (and its y DMA) lands in the kernel tail; rounds 14/15
            # stay bare diagonals so the pipeline thins out at the finish
            fills = {4: [(2, 0)], 5: [(2, 1)], 6: [(2, 2)], 7: [(2, 3)],
                     8: [(3, 0), (3, 1)], 9: [(3, 2), (3, 3)],
                     10: [(3, 4)], 11: [(3, 5)], 12: [(3, 6)], 13: [(3, 7)]}
            for c in range(16):
                pd = c // 4
                for f in fills.get(c, []):
                    add_front(*f, [])
                add_front(pd, c, [chunk_hook(c)])
                if pd + 1 < NPAIR:
                    add_front(pd + 1, c, [])

            def run_hooks(h):
                if h is None:
                    return
                if callable(h):
                    h()
                else:
                    for hh in h:
                        hh()

            av_done = {p: 0 for p in range(NPAIR)}
            lo_done = {p: 0 for p in range(NPAIR)}

            def do_av(item):
                pp, ss, aa = item
                emit_sg_av(pp, ss, o_tiles[pp], aa)
                nssg = 4 * pp + 4
                av_done[pp] += 1
                if ss <= 4 * pp + 1:
                    lo_done[pp] += 1
                    if lo_done[pp] == 4 * pp + 2:
                        emit_norm(pp, 0, o_tiles[pp])
                if av_done[pp] == nssg:
                    grp_tiles.pop(pp, None)
                    emit_norm(pp, 1, o_tiles[pp])

            pending = []      # (p, s, a_tile) awaiting attn@V, lag 3
            for (p, s, hooks) in fronts:
                run_hooks(hooks)
                a = emit_sg_front(p, s)
                pending.append((p, s, a))
                if len(pending) > 3:
                    do_av(pending.pop(0))
            for item in pending:
                do_av(item)

    nc.compile()
    return nc


def _host_prep(x, Wq, bq, Wk, bk, Wv, bv):
    import ml_dtypes
    x = np.ascontiguousarray(np.asarray(x, np.float32))
    xc = np.zeros((CPAD, NTOK), np.float32)
    xc[:259] = x.reshape(259, NTOK)
    t = np.arange(8, dtype=np.float32) / 8 - 0.5
    h = np.arange(32, dtype=np.float32) / 32 - 0.5
    w = np.arange(32, dtype=np.float32) / 32 - 0.5
    pe = np.zeros((3, 8, 32, 32), np.float32)
    pe[0] = t[:, None, None]
    pe[1] = h[None, :, None]
    pe[2] = w[None, None, :]
    xc[259:262] = pe.reshape(3, NTOK)
    xc[262] = 1.0
    ww = np.zeros((CPAD, CW), np.float32)
    for b in range(4):
        ww[:262, 32 * b:32 * b + KC] = np.asarray(Wq, np.float32).T * 16.0
        ww[262, 32 * b:32 * b + KC] = np.asarray(bq, np.float32) * 16.0
    ww[:262, WKO:WKO + KC] = np.asarray(Wk, np.float32).T * 16.0
    ww[262, WKO:WKO + KC] = np.asarray(bk, np.float32) * 16.0
    ww[:262, WVO:WVO + VC] = np.asarray(Wv, np.float32).T
    ww[262, WVO:WVO + VC] = np.asarray(bv, np.float32)
    ww[262, WVO + VC] = 1.0
    f8 = ml_dtypes.float8_e4m3
    b16 = ml_dtypes.bfloat16
    xb0 = np.ascontiguousarray(xc[0:384, 0:512].astype(b16))
    wvb = np.ascontiguousarray(ww[0:384, WVO:WVO + VSTR].astype(b16))
    return (np.ascontiguousarray(xc.astype(f8)),
            np.ascontiguousarray(ww.astype(f8)), xb0, wvb)


def _mask_aux(m):
    """Causal mask tiles for core m. mask[i, d, u, n] answers: is key
    128*(16p+2d+u... (kb'th block, partition i) strictly below query column
    n of the d-th diagonal super-... -- same tensor the device formerly
    computed as (n - 16u - 32d) > (i - m)/8."""
    import ml_dtypes
    i = np.arange(P, dtype=np.float32)[:, None, None, None]
    d = np.arange(8, dtype=np.float32)[None, :, None, None]
    u = np.arange(2, dtype=np.float32)[None, None, :, None]
    n = np.arange(256, dtype=np.float32)[None, None, None, :]
    mask = ((n - 16 * u - 32 * d) > (i - m) / 8.0).astype(np.float32)
    # keep only the 64-col staircase window [64*(dloc//2), +64) per dloc
    win = np.stack([mask[:, dl, :, 64 * (dl // 2):64 * (dl // 2) + 64]
                    for dl in range(8)], axis=1)
    m8 = np.ascontiguousarray(
        win.reshape(P, 8 * 2 * 64).astype(ml_dtypes.float8_e4m3))
    m0 = np.ascontiguousarray(win[:, 0:2].astype(ml_dtypes.bfloat16))
    return m8, m0


def kernel(x, Wq, bq, Wk, bk, Wv, bv):
    global LAST_RESULTS
    from concourse.bass_utils import run_bass_kernel_spmd

    if "nc" not in _CACHE:
        _CACHE["nc"] = _build_bass()
    nc = _CACHE["nc"]

    xc, ww, xb0, wvb = _host_prep(x, Wq, bq, Wk, bk, Wv, bv)
    in_maps = []
    for m in range(NCORES):
        m8, m0 = _mask_aux(m)
        in_maps.append({
            "xq": np.ascontiguousarray(xc[:, m::8]),
            "xkv": xc,
            "ww": ww,
            "mask8": m8,
            "mask0": m0,
            "xb0": xb0,
            "wvb": wvb,
        })

    res = run_bass_kernel_spmd(
        nc, in_maps, core_ids=list(range(NCORES)),
        trace=bool(int(os.environ.get("KBENCH_TRACE", "0"))),
    )
    LAST_RESULTS = res

    out = np.zeros((VC, NQ, NCORES), np.float32)
    for m in range(NCORES):
        out[:, :, m] = res.results[m]["y"].T
    return out.reshape(1, VC, 8, 32, 32)


# revision 84
# speedup vs baseline: 1.0056x; 1.0056x over previous
"""Trainium2 Bass kernel for nn_AttentionBlock (causal single-head attention,
8192 tokens, qk-dim 16, v-dim 128, 1x1-conv projections with positional enc).

Sharding: striped query-parallel over 8 cores. Core m owns query tokens
{m, m+8, ..., m+8184} (1024 queries) -- perfectly balanced causal work AND an
identical instruction stream on every core (required: one NEFF, SPMD).

v2: fp8(e4m3) datapath for the projections and attn@V.
  - x and the 1x1-conv weights are shipped in fp8e4m3; Wq/Wk are scaled x16
    host-side (their native ~0.06 range would hit e4m3 denormals) and the
    compensating 1/256 -- together with the 1/sqrt(259) score scale -- is
    folded into the exp activation's free affine `scale`.
  - Projections contract 264 channels as one DoubleRow matmul over the two
    128-channel subtiles (0.5 cyc/row) plus a regular fp8 tail matmul over
    the 8 remaining rows (pos-enc, ones/bias, pad).
  - exp output is written as fp8 directly by ScalarE; attn@V runs DoubleRow
    (two 128-key blocks per matmul, 256-key contraction), with V stored fp8
    at stride VSTR=144 (DoubleRow weight-step must be 16B-aligned).
  - Scores stay bf16/fp32 (S matmuls bf16, PSUM fp32, exp on ScalarE).
  - Precision guard: super-group 0 of pair 0 (key blocks 0..3 x the first
    256 queries, i.e. every query's first 512 keys on this core) runs the
    old bf16 path -- small-softmax rows (few keys) would otherwise expose
    raw fp8 V quantization in the output.
  - exp is batched 2 score-groups per activation instruction (PSUM source
    spanning 2 banks) to amortize ScalarE's fixed per-instruction cost;
    the causal column cut is applied at super-group granularity and the
    multiplicative mask (fp8, generated on-device) runs on GPSIMD.
  - o_t accumulates lo+hi query-subblocks in ONE PSUM bank as a single
    accumulation group (start=True only on the pair's very first attn@V
    matmul -- a start mid-bank would clear the sibling's has_written bits).
"""

import os
import numpy as np

P = 128
NTOK = 8192
KC, VC = 16, 128
NCORES = 8
NQ = NTOK // NCORES       # 1024 queries per core
QSUBS = NQ // P           # 8
NPAIR = QSUBS // 2        # 4 query-subblock pairs
VSTR = 144                # fp8 V row stride: 128 v + 1 ones + 15 zero pad
CCH = 264                 # channels: 259 x + 3 pos + 1 ones + 1 zero pad
CPAD = 512                # channel rows padded to 4 x 128 subtiles (zeros)
NGRP = NPAIR              # 4 DMA groups of 2048 tokens (4 chunks of 512)
WQR = 128                 # ww cols 0:128   = Wq(x16) replicated at 0/32/64/96
WKO = 128                 # ww cols 128:144 = Wk(x16)
WVO = 144                 # ww cols 144:288 = Wv | ones | zero pad
CW = 288
SCALE = float(1.0 / (256.0 * np.sqrt(259.0)))

LAST_RESULTS = None       # BassKernelResults of the most recent run (for test.py)

_CACHE = {}


def _build_bass():
    import concourse.mybir as mybir
    import concourse.tile as tile
    from concourse import bacc

    f32 = mybir.dt.float32
    bf16 = mybir.dt.bfloat16
    fp8 = mybir.dt.float8e4
    AF = mybir.ActivationFunctionType
    ALU = mybir.AluOpType
    DR = mybir.MatmulPerfMode.DoubleRow

    nc = bacc.Bacc("TRN2", target_bir_lowering=False, debug=False,
                   num_devices=NCORES)

    xq_d = nc.dram_tensor("xq", [CPAD, NQ], fp8, kind="ExternalInput").ap()
    xkv_d = nc.dram_tensor("xkv", [CPAD, NTOK], fp8, kind="ExternalInput").ap()
    ww_d = nc.dram_tensor("ww", [CPAD, CW], fp8, kind="ExternalInput").ap()
    mask_d = nc.dram_tensor("mask8", [P, 8 * 2 * 64], fp8,
                            kind="ExternalInput").ap()
    xb0_d = nc.dram_tensor("xb0", [384, 512], bf16, kind="ExternalInput").ap()
    wvb_d = nc.dram_tensor("wvb", [384, VSTR], bf16,
                           kind="ExternalInput").ap()
    mask0_d = nc.dram_tensor("mask0", [P, 2, 2, 64], bf16,
                             kind="ExternalInput").ap()
    y_d = nc.dram_tensor("y", [NQ, VC], f32, kind="ExternalOutput").ap()

    with tile.TileContext(nc) as tc:
        with (
            tc.tile_pool(name="const", bufs=1) as const,
            tc.tile_pool(name="xpool", bufs=3) as xpool,
            tc.tile_pool(name="work", bufs=5) as work,
            tc.tile_pool(name="small", bufs=8) as small,
            tc.tile_pool(name="ps_s", bufs=2, space="PSUM") as ps_s,
            tc.tile_pool(name="ps_o", bufs=2, space="PSUM") as ps_o,
            tc.tile_pool(name="ps_kq", bufs=1, space="PSUM") as ps_kq,
            tc.tile_pool(name="ps_v", bufs=1, space="PSUM") as ps_v,
        ):
          # ---- body (emitted KREPEAT times for device-time measurement) ----
          for _rep in range(int(os.environ.get("KREPEAT", "1"))):
            # ---- persistent SBUF tensors ----
            ww_sb = const.tile([P, 4, CW], fp8)
            xq_sb = const.tile([P, 4, NQ], fp8)
            mask_sb = const.tile([P, 8, 2, 64], fp8)
            mask0_sb = const.tile([P, 2, 2, 64], bf16)
            qt_sb = const.tile([P, NQ], bf16)
            kt_sb = const.tile([P, NGRP, 512], bf16)
            v_sb = const.tile([P, 32, 2, VSTR], fp8)
            v0_sb = const.tile([P, 4, VSTR], bf16)
            xb0_sb = const.tile([P, 3, 512], bf16)
            wvb_sb = const.tile([P, 3, VSTR], bf16)

            # ---- DMA ordering (all SP-issued, 565ns per issue): everything
            # the first exp needs goes first -- ww, xq half 0, token group 0
            # piece 0 -- then the rest in first-use order. The channel axis
            # is zero-padded host-side to 4x128 subtiles so every tensor
            # loads with ONE descriptor set and the tail contraction can run
            # DoubleRow. Masks come precomputed from the host (cheaper as
            # DMA bytes than as DVE/Pool compute).
            nc.sync.dma_start(ww_sb[:],
                              ww_d.rearrange("(c p) m -> p c m", p=P))

            def dma_xq_piece(c):
                nc.sync.dma_start(
                    xq_sb[:, :, 256 * c:256 * (c + 1)],
                    xq_d[:, 256 * c:256 * (c + 1)]
                    .rearrange("(c p) n -> p c n", p=P))
            wq2 = ww_sb[:, 0:2, 0:WQR]
            wk2 = ww_sb[:, 0:2, WKO:WKO + KC]
            wv2 = ww_sb[:, 0:2, WVO:WVO + VC]
            wqt = ww_sb[:, 2:4, 0:WQR]
            wkt = ww_sb[:, 2:4, WKO:WKO + KC]
            wvt = ww_sb[:, 2:4, WVO:WVO + VC]
            grp_tiles = {}

            def ensure_group_dma(g):
                if g in grp_tiles or g >= NGRP:
                    return
                xg = xpool.tile([P, 4, 2048], fp8, tag="xg", name=f"xg{g}")
                pieces = {0: 4, 1: 2}.get(g, 1)
                for hh in range(pieces):
                    w = 2048 // pieces
                    sl = slice(2048 * g + w * hh, 2048 * g + w * (hh + 1))
                    cs = slice(w * hh, w * (hh + 1))
                    nc.sync.dma_start(
                        xg[:, :, cs],
                        xkv_d[:, sl].rearrange("(c p) n -> p c n", p=P))
                grp_tiles[g] = xg

            # ---- PE p-state warm-up: the cost model ramps the PE clock
            # (0.65 -> 1.2 -> 2.4 GHz) based on how long the engine has been
            # continuously busy; real work only starts ~4.5us in (DMA
            # latency), so a chain of throwaway matmuls on never-read SBUF
            # keeps the PE busy from t=0 and the projections start warm ----
            warm_src = const.tile([P, 512], bf16, name="warm_src")
            nc.gpsimd.memset(warm_src[:], 0.0)
            warm_ps = ps_kq.tile([P, 512], f32, tag="kq", name="warm_ps")
            for _w in range(7):
                nc.tensor.matmul(warm_ps[:], warm_src[:, 0:128],
                                 warm_src[:], start=True, stop=True,
                                 skip_group_check=True)

            # SP issue order: group-0 piece 0 first (gates K0 -> first exp),
            # then xq piece 0 (gates Q proj), then first-use order
            xg0 = xpool.tile([P, 4, 2048], fp8, tag="xg", name="xg0")
            nc.sync.dma_start(
                xg0[:, :, 0:512],
                xkv_d[:, 0:512].rearrange("(c p) n -> p c n", p=P))
            grp_tiles[0] = xg0
            dma_xq_piece(0)
            nc.sync.dma_start(
                xg0[:, :, 512:1024],
                xkv_d[:, 512:1024].rearrange("(c p) n -> p c n", p=P))
            nc.sync.dma_start(mask0_sb[:], mask0_d)
            dma_xq_piece(1)
            nc.sync.dma_start(
                xg0[:, :, 1024:1536],
                xkv_d[:, 1024:1536].rearrange("(c p) n -> p c n", p=P))
            nc.sync.dma_start(
                xg0[:, :, 1536:2048],
                xkv_d[:, 1536:2048].rearrange("(c p) n -> p c n", p=P))
            nc.sync.dma_start(xb0_sb[:],
                              xb0_d.rearrange("(c p) n -> p c n", p=P))
            nc.sync.dma_start(wvb_sb[:],
                              wvb_d.rearrange("(c p) n -> p c n", p=P))
            dma_xq_piece(2)
            dma_xq_piece(3)
            nc.sync.dma_start(
                mask_sb[:], mask_d.rearrange("p (d u n) -> p d u n", d=8, u=2))
            ensure_group_dma(1)
            # softmax-denominator ones column of every V tile (col 128)
            nc.gpsimd.memset(v_sb[:, :, :, VC:VC + 1], 1.0)
            nc.gpsimd.memset(v0_sb[:, :, VC:VC + 1], 1.0)

            # ---- ACT table warm-up: load the exp table set while the input
            # DMAs stream in (the real first exp would otherwise eat ~1.3us
            # on the critical path) ----
            warm_sb = small.tile([P, 1], f32, tag="recip")
            nc.scalar.activation(warm_sb[:], ww_sb[:, 0, 0:1], AF.Exp)

            # ---- Q projection: qt [16, 1024] replicated at partition bases
            # 0/32/64/96 (Wq is host-replicated into 4 column blocks).
            # Half 1 (query cols 512:1024, first needed by pair 2) is
            # deferred to pair 1 so it never gates pair-0's K projection ----
            def emit_q_proj(c):
                # pieces 0/1 borrow the ps_o ring (its o-tiles see no writes
                # until attn@V several fronts later), keeping the single-
                # buffer ps_v ring free for V0 -- otherwise V0's matmuls
                # queue behind the qt0 copy at the PE FIFO head
                csl = slice(256 * c, 256 * (c + 1))
                pool = ps_o if c < 2 else ps_v
                q_ps = pool.tile([P, 2, VSTR], f32, tag="o" if c < 2 else "v",
                                 name=f"qps{c}")
                qv = q_ps[:].rearrange("p a b -> p (a b)")[:, 0:256]
                nc.tensor.matmul(
                    qv, wq2, xq_sb[:, 0:2, csl],
                    start=True, stop=False, perf_mode=DR,
                    skip_group_check=True,
                )
                nc.tensor.matmul(
                    qv, wqt, xq_sb[:, 2:4, csl],
                    start=False, stop=True, perf_mode=DR,
                    skip_group_check=True,
                )
                nc.vector.tensor_copy(qt_sb[:, csl], qv)

            def emit_k_chunk(kq, p, j):
                """K-projection matmuls for chunk 4p+j at partition base 32j.

                DoubleRow requires dst partition base 0 (walrus
                s3d3_mm_valid_dst_partition), so only the j=0 chunk runs DR;
                the others use regular fp8 matmuls per 128-channel subtile.
                """
                xg = grp_tiles[p]
                tsl = slice(j * 512, (j + 1) * 512)
                if j == 0:
                    nc.tensor.matmul(
                        kq[0:KC, :], wk2, xg[:, 0:2, tsl],
                        start=True, stop=False, perf_mode=DR,
                        tile_position=(0, 0), skip_group_check=True,
                    )
                    nc.tensor.matmul(
                        kq[0:KC, :], wkt, xg[:, 2:4, tsl],
                        start=False, stop=True, perf_mode=DR,
                        tile_position=(0, 0), skip_group_check=True,
                    )
                    return
                for ci in range(3):
                    kk = P if ci < 2 else 8
                    nc.tensor.matmul(
                        kq[32 * j:32 * j + KC, :],
                        ww_sb[0:kk, ci, WKO:WKO + KC],
                        xg[0:kk, ci, tsl],
                        start=(ci == 0), stop=(ci == 2),
                        tile_position=(0, 32 * j), skip_group_check=True,
                    )

            def emit_v_chunk(p, j):
                """V-projection for the 4 token tiles of chunk 4p+j.
                4 x 128-col tiles pack into ONE PSUM bank (the softmax-
                denominator ones column is a one-time memset in v_sb, not
                part of the projection), so the whole chunk moves to SBUF
                with a single DVE copy."""
                xg = grp_tiles[p]
                v_ps = ps_v.tile([P, 4, VC], f32, tag="v", name=f"vps{p}{j}")
                for tl in range(4):
                    tsl = slice(j * 512 + tl * P, j * 512 + (tl + 1) * P)
                    nc.tensor.matmul(
                        v_ps[:, tl, :], xg[:, 0:2, tsl], wv2,
                        start=True, stop=False, perf_mode=DR,
                        skip_group_check=True,
                    )
                    nc.tensor.matmul(
                        v_ps[:, tl, :], xg[:, 2:4, tsl], wvt,
                        start=False, stop=True, perf_mode=DR,
                        skip_group_check=True,
                    )
                kk = 8 * p + 2 * j
                # (GPSIMD cannot read PSUM, so this stays on DVE)
                nc.vector.tensor_copy(
                    v_sb[:, kk:kk + 2, :, 0:VC], v_ps[:])

            def emit_v0_proj():
                """bf16 V projection for token tiles 0..3 (the fp8-projected
                V would expose its ~3% error directly in the outputs of the
                first, few-key softmax rows; tiles 0..3 cover every row the
                bf16 attn@V fallback path reads). Ones column comes from a
                memset; 4 x 128-col tiles fit one PSUM bank."""
                v0_ps = ps_v.tile([P, 4, VC], f32, tag="v", name="v0ps")
                for tl in range(4):
                    tsl = slice(tl * P, (tl + 1) * P)
                    for ci in range(3):
                        kk = P if ci < 2 else 8
                        nc.tensor.matmul(
                            v0_ps[:, tl, :], xb0_sb[0:kk, ci, tsl],
                            wvb_sb[0:kk, ci, 0:VC],
                            start=(ci == 0), stop=(ci == 2),
                            skip_group_check=True,
                        )
                nc.vector.tensor_copy(v0_sb[:, :, 0:VC], v0_ps[:])

            def emit_sg_front(p, s):
                """S matmuls + batched exp (+ causal mask for diagonal
                super-groups) for super-group s of pair p; returns the a tile.

                The mask multiply only touches [cut, cut+64): each of the 4
                key blocks has a 16-column staircase window there; above it
                the mask is exactly 1.0, below it the column cut applies.
                """
                nsg = 4 * p + 4
                diag = s >= nsg - 4
                dd = s - (nsg - 4)
                cut = 64 * dd if diag else 0
                bf16_path = (p == 0 and s == 0)
                s_ps = ps_s.tile([P, 2, 2, 256], f32, tag="s")
                for gg in range(2):
                    for u in range(2):
                        kb = 4 * s + 2 * gg + u
                        c = kb // 4
                        b = 32 * (c % 4)
                        klhs = kt_sb[b:b + KC, c // 4,
                                     128 * (kb % 4):128 * (kb % 4) + 128]
                        qrhs = qt_sb[b:b + KC, 256 * p + cut:256 * p + 256]
                        nc.tensor.matmul(
                            s_ps[:, gg, u, cut:256], klhs, qrhs,
                            start=True, stop=True,
                            tile_position=(b, 0),
                        )
                a_dt = bf16 if bf16_path else fp8
                a_sb = work.tile([P, 2, 2, 256], a_dt,
                                 tag="a16" if bf16_path else "a8")
                # attn@V reads fixed [0:128) / [128:256) query windows of a;
                # zero the sub-cut region it would read but exp won't write
                if diag and dd == 1:
                    nc.gpsimd.memset(a_sb[:, :, :, 0:64], 0.0)
                elif diag and dd == 3:
                    nc.gpsimd.memset(a_sb[:, :, :, 128:192], 0.0)
                nc.scalar.activation(a_sb[:, :, :, cut:256],
                                     s_ps[:, :, :, cut:256],
                                     AF.Exp, scale=SCALE)
                if diag:
                    # the shipped mask tiles hold ONLY the 64-col staircase
                    # window [cut, cut+64) of each (dloc, u)
                    msk = mask0_sb if bf16_path else mask_sb[:, 2 * dd:2 * dd + 2]
                    hi = min(256, cut + 64)
                    nc.vector.tensor_mul(a_sb[:, :, :, cut:hi],
                                          a_sb[:, :, :, cut:hi],
                                          msk[:, :, :, 0:hi - cut])
                return a_sb

            def emit_sg_av(p, s, o_t, a_sb):
                """attn@V for super-group s of pair p (fp8 DoubleRow, 2 key
                blocks per matmul; bf16 fallback for pair-0 super-group 0)."""
                nsg = 4 * p + 4
                bf16_path = (p == 0 and s == 0)
                nmm_lo_g = 8 * p + 4      # first group with no lo contribution
                if bf16_path:
                    for gg in range(2):
                        for u in range(2):
                            kb = 2 * gg + u
                            nc.tensor.matmul(
                                o_t[:, 0, :], a_sb[:, gg, u, 0:128],
                                v0_sb[:, kb, :],
                                start=(kb == 0), stop=False,
                                skip_group_check=True,
                            )
                            nc.tensor.matmul(
                                o_t[:, 1, :], a_sb[:, gg, u, 128:256],
                                v0_sb[:, kb, :],
                                start=False, stop=False,
                                skip_group_check=True,
                            )
                    return
                for gg in range(2):
                    g = 2 * s + gg
                    vr = v_sb[:, g, :, :]
                    if g < nmm_lo_g:
                        nc.tensor.matmul(
                            o_t[:, 0, :], a_sb[:, gg, :, 0:128], vr,
                            start=(p > 0 and s == 0 and gg == 0),
                            stop=False, perf_mode=DR,
                            skip_group_check=True,
                        )
                    last = (s == last_av_s[p] and gg == 1)
                    nc.tensor.matmul(
                        o_t[:, 1, :], a_sb[:, gg, :, 128:256], vr,
                        start=False, stop=last, perf_mode=DR,
                        skip_group_check=True,
                    )

            def emit_norm(p, half, o_t):
                qs = 2 * p + half
                recip_sb = small.tile([P, 1], f32, tag="recip")
                if p == 0:
                    # only pair 0 can see an all-masked row (denominator 0)
                    nc.vector.tensor_scalar_add(
                        recip_sb[:], o_t[:, half, VC:VC + 1], 1e-30)
                    nc.vector.reciprocal(recip_sb[:], recip_sb[:])
                else:
                    nc.vector.reciprocal(recip_sb[:], o_t[:, half, VC:VC + 1])
                on_sb = small.tile([P, VC], f32, tag="on")
                nc.vector.tensor_scalar_mul(
                    on_sb[:], o_t[:, half, 0:VC], recip_sb[:])
                nc.sync.dma_start(y_d[qs * P:(qs + 1) * P, :], on_sb[:])

            # ---- chunk-major wavefront schedule. Super-group (p, s) needs
            # exactly global token-chunk s (keys 512s..512s+511), which is
            # produced by pair s//4's projection. So: 16 rounds, one chunk
            # each; after producing chunk c run its diagonal super-group
            # (c//4, c), its off-diagonal partner (c//4+1, c), and one
            # backlog "filler" super-group of a later pair whose chunk is
            # long since resident. This keeps ScalarE fed with a uniform
            # 2-3 exps per round (the old pair-sequential order crammed
            # pair 3's 16 super-groups at the end) and caps live o_t
            # accumulators at 2 (ps_o bufs=2; ps_v only needs 1 buffer
            # since V chunks are now one per round). attn@V still trails
            # by 3 fronts; the lo/hi norm trigger conditions are unchanged
            # because each pair's last lo / last av front is still its
            # s == nsg-3 / nsg-1 diagonal. ----
            o_tiles = {}
            fronts = []       # (p, s, pre_hook)
            last_av_s = {0: 3, 1: 7, 2: 11, 3: 15}  # last-PROCESSED sg per pair

            def chunk_hook(c):
                pc, j = c // 4, c % 4
                def hook():
                    if j == 0:
                        ensure_group_dma(pc + 1)
                        ensure_group_dma(pc + 2)
                        grp_tiles[("kq", pc)] = ps_kq.tile(
                            [P, 512], f32, tag="kq", name=f"kq{pc}")
                    kq = grp_tiles[("kq", pc)]
                    emit_k_chunk(kq, pc, j)
                    nc.vector.tensor_copy(
                        kt_sb[32 * j:32 * j + KC, pc, :],
                        kq[32 * j:32 * j + KC, :])
                    if c == 0:
                        emit_q_proj(0)
                    emit_v_chunk(pc, j)
                    if c == 0:
                        emit_q_proj(1)
                    elif c == 1:
                        emit_v0_proj()
                    elif c == 2:
                        emit_q_proj(2)
                    elif c == 6:
                        emit_q_proj(3)
                return hook

            def alloc_o(p):
                def hook():
                    o_tiles[p] = ps_o.tile([P, 2, VSTR], f32, tag="o",
                                           name=f"o{p}")
                return hook

            seen_pairs = set()

            def add_front(p, s, hooks):
                if p not in seen_pairs:
                    seen_pairs.add(p)
                    hooks = list(hooks) + [alloc_o(p)]
                fronts.append((p, s, hooks))

            # backlog fillers, front-loaded: pair-3's fills are all lo-half
            # super-groups, so they must finish well before the end or the
            # lo norm (and its y DMA) lands in the kernel tail; rounds 14/15
            # stay bare diagonals so the pipeline thins out at the finish
            fills = {4: [(2, 0)], 5: [(2, 1)], 6: [(2, 2)], 7: [(2, 3)],
                     8: [(3, 0), (3, 1)], 9: [(3, 2), (3, 3)],
                     10: [(3, 4)], 11: [(3, 5)], 12: [(3, 6)], 13: [(3, 7)]}
            for c in range(16):
                pd = c // 4
                for f in fills.get(c, []):
                    add_front(*f, [])
                add_front(pd, c, [chunk_hook(c)])
                if pd + 1 < NPAIR:
                    add_front(pd + 1, c, [])

            def run_hooks(h):
                if h is None:
                    return
                if callable(h):
                    h()
                else:
                    for hh in h:
                        hh()

            av_done = {p: 0 for p in range(NPAIR)}
            lo_done = {p: 0 for p in range(NPAIR)}

            def do_av(item):
                pp, ss, aa = item
                emit_sg_av(pp, ss, o_tiles[pp], aa)
                nssg = 4 * pp + 4
                av_done[pp] += 1
                if ss <= 4 * pp + 1:
                    lo_done[pp] += 1
                    if lo_done[pp] == 4 * pp + 2:
                        emit_norm(pp, 0, o_tiles[pp])
                if av_done[pp] == nssg:
                    grp_tiles.pop(pp, None)
                    emit_norm(pp, 1, o_tiles[pp])

            pending = []      # (p, s, a_tile) awaiting attn@V, lag 3
            for (p, s, hooks) in fronts:
                run_hooks(hooks)
                a = emit_sg_front(p, s)
                pending.append((p, s, a))
                if len(pending) > 3:
                    do_av(pending.pop(0))
            for item in pending:
                do_av(item)

    nc.compile()
    return nc


def _host_prep(x, Wq, bq, Wk, bk, Wv, bv):
    import ml_dtypes
    x = np.ascontiguousarray(np.asarray(x, np.float32))
    xc = np.zeros((CPAD, NTOK), np.float32)
    xc[:259] = x.reshape(259, NTOK)
    t = np.arange(8, dtype=np.float32) / 8 - 0.5
    h = np.arange(32, dtype=np.float32) / 32 - 0.5
    w = np.arange(32, dtype=np.float32) / 32 - 0.5
    pe = np.zeros((3, 8, 32, 32), np.float32)
    pe[0] = t[:, None, None]
    pe[1] = h[None, :, None]
    pe[2] = w[None, None, :]
    xc[259:262] = pe.reshape(3, NTOK)
    xc[262] = 1.0
    ww = np.zeros((CPAD, CW), np.float32)
    for b in range(4):
        ww[:262, 32 * b:32 * b + KC] = np.asarray(Wq, np.float32).T * 16.0
        ww[262, 32 * b:32 * b + KC] = np.asarray(bq, np.float32) * 16.0
    ww[:262, WKO:WKO + KC] = np.asarray(Wk, np.float32).T * 16.0
    ww[262, WKO:WKO + KC] = np.asarray(bk, np.float32) * 16.0
    ww[:262, WVO:WVO + VC] = np.asarray(Wv, np.float32).T
    ww[262, WVO:WVO + VC] = np.asarray(bv, np.float32)
    ww[262, WVO + VC] = 1.0
    f8 = ml_dtypes.float8_e4m3
    b16 = ml_dtypes.bfloat16
    xb0 = np.ascontiguousarray(xc[0:384, 0:512].astype(b16))
    wvb = np.ascontiguousarray(ww[0:384, WVO:WVO + VSTR].astype(b16))
    return (np.ascontiguousarray(xc.astype(f8)),
            np.ascontiguousarray(ww.astype(f8)), xb0, wvb)


def _mask_aux(m):
    """Causal mask tiles for core m. mask[i, d, u, n] answers: is key
    128*(16p+2d+u... (kb'th block, partition i) strictly below query column
    n of the d-th diagonal super-... -- same tensor the device formerly
    computed as (n - 16u - 32d) > (i - m)/8."""
    import ml_dtypes
    i = np.arange(P, dtype=np.float32)[:, None, None, None]
    d = np.arange(8, dtype=np.float32)[None, :, None, None]
    u = np.arange(2, dtype=np.float32)[None, None, :, None]
    n = np.arange(256, dtype=np.float32)[None, None, None, :]
    mask = ((n - 16 * u - 32 * d) > (i - m) / 8.0).astype(np.float32)
    # keep only the 64-col staircase window [64*(dloc//2), +64) per dloc
    win = np.stack([mask[:, dl, :, 64 * (dl // 2):64 * (dl // 2) + 64]
                    for dl in range(8)], axis=1)
    m8 = np.ascontiguousarray(
        win.reshape(P, 8 * 2 * 64).astype(ml_dtypes.float8_e4m3))
    m0 = np.ascontiguousarray(win[:, 0:2].astype(ml_dtypes.bfloat16))
    return m8, m0


def kernel(x, Wq, bq, Wk, bk, Wv, bv):
    global LAST_RESULTS
    from concourse.bass_utils import run_bass_kernel_spmd

    if "nc" not in _CACHE:
        _CACHE["nc"] = _build_bass()
    nc = _CACHE["nc"]

    xc, ww, xb0, wvb = _host_prep(x, Wq, bq, Wk, bk, Wv, bv)
    in_maps = []
    for m in range(NCORES):
        m8, m0 = _mask_aux(m)
        in_maps.append({
            "xq": np.ascontiguousarray(xc[:, m::8]),
            "xkv": xc,
            "ww": ww,
            "mask8": m8,
            "mask0": m0,
            "xb0": xb0,
            "wvb": wvb,
        })

    res = run_bass_kernel_spmd(
        nc, in_maps, core_ids=list(range(NCORES)),
        trace=bool(int(os.environ.get("KBENCH_TRACE", "0"))),
    )
    LAST_RESULTS = res

    out = np.zeros((VC, NQ, NCORES), np.float32)
    for m in range(NCORES):
        out[:, :, m] = res.results[m]["y"].T
    return out.reshape(1, VC, 8, 32, 32)
